# revision 1
# baseline (speedup 1.0000x reference)
"""Sigmoid-gated attention on 8 TRN2 NeuronCores.

Reference computation (per full problem):
    Q = q @ Wq + bq; K = x @ Wk + bk; V = x @ Wv + bv
    out = sigmoid((Q @ K.T) / sqrt(d)) @ V

Sharding: rows of q (query sequence) are split across the 8 cores; x and all
weights are replicated. Each core computes its 512-query slice independently
(no collectives).

Algebraic restructure (the key optimization): K and V are never materialized.
    S   = Q @ K.T = q @ (Wq @ Wk.T) @ x.T  + rank-1 bias terms
    out = G @ V   = (G @ x) @ Wv           + rowsum(G) x bv,   G = sigmoid(S/32)
The weight-weight product M = Wq @ Wk.T is folded on the host in f32. This
cuts per-core device FLOPs from ~27 GF (replicated K/V projections) to
~11.8 GF, exactly 1/8 of the algorithm's total.

Device dataflow per core (all matmuls bf16 with f32 PSUM accumulation; the
i-dim (512 local queries) is the moving free dim everywhere; every operand
is consumed in its natural layout thanks to host-side transposes):
    A: AT[c,i]   = sum_c' M[c',c]^T qT[c',i]        (then * 1/32, -> bf16)
    B: ST[j,i]   = sum_c  xT[c,j]^T AT[c,i]  (+ ck[i])
       GT[j,i]   = sigmoid(ST + sbias[j])           (-> bf16)
       rs[i]    += ones[j]^T GT[j,i]                (only if bv != 0)
    C: GxT[c,i]  = sum_j  x[j,c]^T GT[j,i]          (-> bf16)
    D: OT[f,i]   = sum_c  Wv[c,f]^T GxT[c,i] (+ bv[f] rs[i]) -> f32 out
Bias terms (bq/bk/bv are zero in this problem's inputs) are folded to host
vectors and only compiled in when nonzero, so the general case stays exact.
"""

import sys

for _p in ("/opt/trn_rl_repo", "/opt/pypackages"):
    if _p not in sys.path:
        sys.path.append(_p)

import numpy as np
import ml_dtypes

LQ, LK, CIN, COUT = 4096, 4096, 1024, 1024
N_CORES = 8
IQ = LQ // N_CORES  # 512 queries per core = moving free dim
P = 128
NCT = CIN // P  # 8 tiles along any 1024 feature dim
NJ = LK // P  # 32 key tiles
SCALE = 1.0 / np.sqrt(np.float32(COUT))
BF16 = ml_dtypes.bfloat16

_cache = {}
_last_in_maps = None


def _build(use_ck, use_sbias, use_bv):
    import concourse.tile as tile
    from concourse import bacc, mybir
    from contextlib import ExitStack

    bf = mybir.dt.bfloat16
    f32 = mybir.dt.float32

    nc = bacc.Bacc("TRN2", target_bir_lowering=False, debug=False, num_devices=N_CORES)

    qT = nc.dram_tensor("qT", [CIN, IQ], bf, kind="ExternalInput")
    Mw = nc.dram_tensor("Mw", [CIN, CIN], bf, kind="ExternalInput")
    xT = nc.dram_tensor("xT", [CIN, LK], bf, kind="ExternalInput")
    xN = nc.dram_tensor("xN", [LK, CIN], bf, kind="ExternalInput")
    Wv = nc.dram_tensor("Wv", [CIN, COUT], bf, kind="ExternalInput")
    sb = nc.dram_tensor("sbias", [P, NJ], f32, kind="ExternalInput") if use_sbias else None
    ck = nc.dram_tensor("ck", [1, IQ], bf, kind="ExternalInput") if use_ck else None
    bv = nc.dram_tensor("bv", [1, COUT], bf, kind="ExternalInput") if use_bv else None
    ones = (
        nc.dram_tensor("ones", [P, P], bf, kind="ExternalInput")
        if (use_ck or use_bv)
        else None
    )
    outT = nc.dram_tensor("outT", [COUT, IQ], f32, kind="ExternalOutput")

    with tile.TileContext(nc) as tc, ExitStack() as ctx:
        res = ctx.enter_context(tc.tile_pool(name="res", bufs=1))
        xs = ctx.enter_context(tc.tile_pool(name="xs", bufs=12))
        outp = ctx.enter_context(tc.tile_pool(name="outp", bufs=4))

        # Resident SBUF tensors: tile chunks packed along the free dim.
        m_sb = res.tile([P, NCT * CIN], bf, tag="m")  # chunk cp: M[128cp:+128, :]
        qt_sb = res.tile([P, NCT * IQ], bf, tag="qt")  # chunk cp: qT[128cp:+128, :]
        xt_sb = res.tile([P, NCT * LK], bf, tag="xt")  # chunk c: xT[128c:+128, :]
        wv_sb = res.tile([P, NCT * COUT], bf, tag="wv")  # chunk c: Wv[128c:+128, :]
        at_sb = res.tile([P, NCT * IQ], bf, tag="at")  # chunk c: AT tile [128, 512]
        g_sb = res.tile([P, NJ * IQ], bf, tag="g")  # chunk j: GT tile [128, 512]
        gx_sb = res.tile([P, NCT * IQ], bf, tag="gx")  # chunk c: GxT tile [128, 512]

        # cp=0 chunks first: phase A's first matmul depends only on qT c0 +
        # the first 128-column slice of M c0 (~160KB), DMA'd ahead of the rest
        nc.sync.dma_start(qt_sb[:, 0:IQ], qT.ap()[0:P, :])
        nc.sync.dma_start(m_sb[:, 0:P], Mw.ap()[0:P, 0:P])
        nc.sync.dma_start(m_sb[:, P:CIN], Mw.ap()[0:P, P:CIN])
        for cp in range(1, NCT):
            nc.sync.dma_start(
                qt_sb[:, cp * IQ : (cp + 1) * IQ], qT.ap()[cp * P : (cp + 1) * P, :]
            )
            nc.sync.dma_start(
                m_sb[:, cp * CIN : (cp + 1) * CIN], Mw.ap()[cp * P : (cp + 1) * P, :]
            )
        # xT loaded in j-blocks of 1024 so phase B's early j-tiles are ready
        # fast; smallish per-(c,jb) pieces keep A's critical loads competitive
        # in the SDMA packet round-robin
        JB = 1024
        for jb in range(LK // JB):
            for c in range(NCT):
                nc.sync.dma_start(
                    xt_sb[:, c * LK + jb * JB : c * LK + (jb + 1) * JB],
                    xT.ap()[c * P : (c + 1) * P, jb * JB : (jb + 1) * JB],
                )
        nc.sync.dma_start(
            wv_sb.rearrange("p (c f) -> p c f", f=COUT),
            Wv.ap().rearrange("(c p) f -> p c f", p=P),
        )

        if use_sbias:
            sb_sb = res.tile([P, NJ], f32, tag="sb")
            nc.sync.dma_start(sb_sb[:], sb.ap()[:])
        if use_ck:
            ck_sb = res.tile([1, IQ], bf, tag="ck")
            nc.sync.dma_start(ck_sb[:], ck.ap()[:])
        if use_bv:
            bv_sb = res.tile([1, COUT], bf, tag="bv")
            nc.sync.dma_start(bv_sb[:], bv.ap()[:])
        if ones is not None:
            ones_sb = res.tile([P, P], bf, tag="ones")
            nc.sync.dma_start(ones_sb[:], ones.ap()[:])

        # One PSUM pool with a single shared tag for every [128, 512] f32
        # accumulator. Slot recycling gives per-slot deps between phases
        # instead of pool-boundary barriers (PE order already serializes the
        # phases; the allocator must not add coarser waits).
        nbank = 8
        with tc.tile_pool(name="ps", bufs=1, space="PSUM") as ps:
            # Phase A (cp outer): first matmuls need only the cp=0 DMA chunks.
            # AT[ct][c,i] accumulates over cp in its own bank.
            a_ps = [
                ps.tile([P, IQ], f32, tag="mm", bufs=nbank, name=f"a_ps{ct}")
                for ct in range(NCT)
            ]
            for cp in range(NCT):
                for ct in range(NCT):
                    nc.tensor.matmul(
                        a_ps[ct][:],
                        m_sb[:, cp * CIN + ct * P : cp * CIN + (ct + 1) * P],
                        qt_sb[:, cp * IQ : (cp + 1) * IQ],
                        start=(cp == 0),
                        stop=(cp == NCT - 1),
                    )
            for ct in range(NCT):
                # fold in the sigmoid scale while casting to bf16
                nc.scalar.mul(at_sb[:, ct * IQ : (ct + 1) * IQ], a_ps[ct][:], float(SCALE))

            # Phase B: ST -> sigmoid -> GT (+ optional rowsum accumulation)
            for j in range(NJ):
                s_ps = ps.tile([P, IQ], f32, tag="mm", bufs=nbank, name=f"s_ps{j}")
                for c in range(NCT):
                    nc.tensor.matmul(
                        s_ps[:],
                        xt_sb[:, c * LK + j * P : c * LK + (j + 1) * P],
                        at_sb[:, c * IQ : (c + 1) * IQ],
                        start=(c == 0),
                        stop=(c == NCT - 1 and not use_ck),
                    )
                if use_ck:
                    nc.tensor.matmul(
                        s_ps[:], ones_sb[0:1, :], ck_sb[:], start=False, stop=True
                    )
                gt = g_sb[:, j * IQ : (j + 1) * IQ]
                nc.scalar.activation(
                    gt,
                    s_ps[:],
                    mybir.ActivationFunctionType.Sigmoid,
                    bias=sb_sb[:, j : j + 1] if use_sbias else 0.0,
                    scale=1.0,
                )
            # Phase C: GxT[c,i] = sum_j x_chunk[j][:, c*128:+128]^T @ GT[j]
            gx_ps = [
                ps.tile([P, IQ], f32, tag="mm", bufs=nbank, name=f"gx_ps{c}")
                for c in range(NCT)
            ]
            for j in range(NJ):
                x_sb = xs.tile([P, CIN], bf, tag="xj")
                nc.sync.dma_start(x_sb[:], xN.ap()[j * P : (j + 1) * P, :])
                for c in range(NCT):
                    nc.tensor.matmul(
                        gx_ps[c][:],
                        x_sb[:, c * P : (c + 1) * P],
                        g_sb[:, j * IQ : (j + 1) * IQ],
                        start=(j == 0),
                        stop=(j == NJ - 1),
                    )
            # split the 8 drain copies across DVE and ACT to halve the C->D stall
            for c in range(NCT):
                dst = gx_sb[:, c * IQ : (c + 1) * IQ]
                if c % 2 == 0:
                    nc.vector.tensor_copy(dst, gx_ps[c][:])
                else:
                    nc.scalar.copy(dst, gx_ps[c][:])

            # rowsum(G) for the bv rank-1 term (general path only; one extra
            # PSUM slot from the shared tag, after C's accumulators retire)
            if use_bv:
                rs_ps = ps.tile([1, IQ], f32, tag="mm", bufs=nbank, name="rs_ps")
                for j in range(NJ):
                    nc.tensor.matmul(
                        rs_ps[:],
                        ones_sb[:, 0:1],
                        g_sb[:, j * IQ : (j + 1) * IQ],
                        start=(j == 0),
                        stop=(j == NJ - 1),
                    )
                rs_sb = res.tile([1, IQ], bf, tag="rssb")
                nc.vector.tensor_copy(rs_sb[:], rs_ps[:])

            # Phase D: OT[f,i] = sum_c Wv_chunk[c][:, f*128:+128]^T @ GxT[c]
            for ft in range(NCT):
                o_ps = ps.tile([P, IQ], f32, tag="mm", bufs=nbank, name=f"o_ps{ft}")
                for c in range(NCT):
                    nc.tensor.matmul(
                        o_ps[:],
                        wv_sb[:, c * COUT + ft * P : c * COUT + (ft + 1) * P],
                        gx_sb[:, c * IQ : (c + 1) * IQ],
                        start=(c == 0),
                        stop=(c == NCT - 1 and not use_bv),
                    )
                if use_bv:
                    nc.tensor.matmul(
                        o_ps[:],
                        bv_sb[0:1, ft * P : (ft + 1) * P],
                        rs_sb[:],
                        start=False,
                        stop=True,
                    )
                # drain each output tile as two engine-parallel halves so the
                # final tile's copy+store tail is halved
                o_sb = outp.tile([P, IQ], f32, tag="osb")
                h = IQ // 2
                nc.vector.tensor_copy(o_sb[:, 0:h], o_ps[:, 0:h])
                nc.scalar.copy(o_sb[:, h:IQ], o_ps[:, h:IQ])
                nc.sync.dma_start(outT.ap()[ft * P : (ft + 1) * P, 0:h], o_sb[:, 0:h])
                nc.sync.dma_start(outT.ap()[ft * P : (ft + 1) * P, h:IQ], o_sb[:, h:IQ])

    nc.compile()
    return nc


def kernel(q, x, Wq, bq, Wk, bk, Wv, bv):
    from concourse.bass_utils import run_bass_kernel_spmd

    q = np.asarray(q, np.float32)
    x = np.asarray(x, np.float32)
    Wq = np.asarray(Wq, np.float32)
    bq = np.asarray(bq, np.float32)
    Wk = np.asarray(Wk, np.float32)
    bk = np.asarray(bk, np.float32)
    Wv = np.asarray(Wv, np.float32)
    bv = np.asarray(bv, np.float32)

    Mw = Wq @ Wk.T  # [c', c] in f32 on host
    wqbk = Wq @ bk  # ck[i] = (q_i . wqbk + bq.bk) * SCALE  (free-dim bias of S)
    wkbq = Wk @ bq  # sbias[j] = (x_j . wkbq) * SCALE       (partition bias of S)
    bqbk = float(bq @ bk)

    sbias = (x @ wkbq) * SCALE  # [LK] (the bq.bk constant lives in cks)
    use_sbias = bool(np.any(sbias != 0.0))
    cks = (q @ wqbk + bqbk) * SCALE  # [LQ]
    use_ck = bool(np.any(cks != 0.0))
    use_bv = bool(np.any(bv != 0.0))

    key = (use_ck, use_sbias, use_bv)
    if key not in _cache:
        _cache[key] = _build(*key)
    nc = _cache[key]

    common = {
        "Mw": np.ascontiguousarray(Mw).astype(BF16),
        "xT": np.ascontiguousarray(x.T).astype(BF16),
        "xN": np.ascontiguousarray(x).astype(BF16),
        "Wv": np.ascontiguousarray(Wv).astype(BF16),
    }
    if use_sbias:
        common["sbias"] = np.ascontiguousarray(sbias.reshape(NJ, P).T).astype(np.float32)
    if use_bv:
        common["bv"] = bv.reshape(1, COUT).astype(BF16)
    if use_ck or use_bv:
        common["ones"] = np.ones((P, P), BF16)

    in_maps = []
    for c in range(N_CORES):
        m = dict(common)
        m["qT"] = np.ascontiguousarray(q[c * IQ : (c + 1) * IQ].T).astype(BF16)
        if use_ck:
            m["ck"] = cks[c * IQ : (c + 1) * IQ].reshape(1, IQ).astype(BF16)
        in_maps.append(m)

    global _last_in_maps
    _last_in_maps = in_maps
    res = run_bass_kernel_spmd(nc, in_maps, core_ids=list(range(N_CORES)))
    out = np.concatenate(
        [np.asarray(res.results[c]["outT"]).T for c in range(N_CORES)], axis=0
    )
    return np.ascontiguousarray(out, dtype=np.float32)

